# revision 54
# baseline (speedup 1.0000x reference)
"""LogNeuralCDE on 8 NeuronCores, batch-sharded (32 items/core).

Whole Heun scan runs on-device. Math: for each func eval, the Lie-bracket
contraction sum_p c_p (J[i0,i1]-J[i1,i0]) is rewritten as sum_{d,e} C[b,d,e]
J[b,d,e,:] with C antisymmetric, and C is contracted against the tangent
basis X_d = vf rows BEFORE the JVP chain (linearity), so the chain runs on 8
e-tangents and the 512-wide Wvo matmul happens blockwise (8x smaller).
The s-term rides along as a 9th block through the same block-diag matmul.
Per-step C matrices (scaled by dt/denom) are host-built and preloaded.
"""

import numpy as np

N_CORES = 8
N_STEPS = 32
BL = 32          # batch per core
D = 8
H = 64
SIG = 37
LABEL = 10

_PATCHED = False


def _apply_compat_patches():
    """This container's neuronx-cc allows at most ONE sync wait per
    instruction (setupSyncWait: 'Too many sync wait commands').  Two fixes:
    1. TileContext's final drain carries the whole global-clock wait list —
       split it one-wait-per-NoOp before a bare drain.
    2. The tile scheduler emits compute instructions with 2 waits — rewrite
       the BIR JSON just before compile, moving excess waits onto NoOps
       inserted ahead of the instruction on the same engine.
    """
    global _PATCHED
    if _PATCHED:
        return
    _PATCHED = True
    import json
    from concourse import bass2jax, mybir
    from concourse.tile import TileContext
    from concourse.vector_clock import ScopedClock

    def _drain_and_barrier(self, tick_clock, wait_clock):
        nop_inst = self.nc.sync.nop(nofuse=True, hint="tile_final_wait")
        wait_clock.add_sem_waits(
            nop_inst.ins, ScopedClock({None: tick_clock.global_clock}))
        si = nop_inst.ins.sync_info
        waits = list(si.on_wait) if si is not None else []
        if len(waits) > 1:
            si.on_wait = waits[:1]
            for w in waits[1:]:
                extra = self.nc.sync.nop(nofuse=True, hint="tile_final_wait_x")
                extra.ins.sync_info = mybir.SyncInfo(on_wait=[w], on_update=[])
        self.nc.sync.drain()
        self.nc.all_engine_barrier()
        assert self.sems is not None
        popped = self.nc._tile_sem_poison_stack.pop()
        assert popped is self._sem_poison
        self.nc.clear_and_free_semaphores(list(self.sems.allocated().values()))
        self.nc.all_engine_barrier()

    TileContext._drain_and_barrier = _drain_and_barrier

    _orig_cbk = bass2jax.compile_bir_kernel

    def _split_excess_waits(bir_bytes):
        d = json.loads(bir_bytes)
        changed = False
        for fn in d.get("functions", []):
            for blk in fn.get("blocks", []):
                out = []
                ctr = 0
                for inst in blk.get("instructions", []):
                    si = inst.get("sync_info")
                    if si:
                        waits = si.get("on_wait") or []
                        if len(waits) > 1:
                            for w in waits[:-1]:
                                ctr += 1
                                out.append({
                                    "debug": inst.get("debug", 0),
                                    "engine": inst["engine"],
                                    "ins": [], "outs": [],
                                    "name": f"{inst['name']}_xw{ctr}",
                                    "opcode": "NoOp",
                                    "sync_info": {"on_update": [],
                                                  "on_wait": [w]},
                                })
                            si["on_wait"] = waits[-1:]
                            changed = True
                    out.append(inst)
                    if si:
                        ups = si.get("on_update") or []
                        if len(ups) > 1:
                            si["on_update"] = ups[:1]
                            for u in ups[1:]:
                                ctr += 1
                                out.append({
                                    "debug": inst.get("debug", 0),
                                    "engine": inst["engine"],
                                    "ins": [], "outs": [],
                                    "name": f"{inst['name']}_xu{ctr}",
                                    "opcode": "NoOp",
                                    "sync_info": {"on_update": [u],
                                                  "on_wait": []},
                                })
                            changed = True
                blk["instructions"] = out
        return json.dumps(d).encode() if changed else bir_bytes

    def _patched_cbk(bir_bytes, *a, **k):
        return _orig_cbk(_split_excess_waits(bir_bytes), *a, **k)

    bass2jax.compile_bir_kernel = _patched_cbk


# ---------------------------------------------------------------- host math
def _host_prep(ts, intervals, logsig, x0, pairs, W1, b1):
    f32 = np.float32
    B = x0.shape[0]
    t0, t1 = f32(ts[0]), f32(ts[-1])
    dt = f32((t1 - t0) / N_STEPS)
    times = (t0 + dt * np.arange(N_STEPS, dtype=f32)).astype(f32)
    nI = intervals.shape[0] - 1

    def eidx(t):
        return int(np.clip(np.searchsorted(intervals, t), 1, nI))

    idx1 = [eidx(times[k]) for k in range(N_STEPS)]
    idx2 = [eidx(f32(times[k] + dt)) for k in range(N_STEPS)]
    uniq = sorted(set(idx1) | set(idx2))
    pos = {ix: i for i, ix in enumerate(uniq)}
    sl1 = [pos[i] for i in idx1]
    sl2 = [pos[i] for i in idx2]
    nS = len(uniq)

    i0 = pairs[:, 0].astype(np.int64) - 1
    i1 = pairs[:, 1].astype(np.int64) - 1

    # Cblk[core, s, chain, 128, 144]: per 16-batch chain, rows (d, b16),
    # cols e*16+b16 | s-term at 128+b16.  All 8 d fit in 128 partitions.
    cblk = np.zeros((N_CORES, nS, 2, 128, 144), np.float32)
    b = np.arange(BL)
    b16 = np.arange(16)
    for c in range(N_CORES):
        lsg = logsig[c * BL:(c + 1) * BL]          # [BL, nI, SIG]
        for si, ix in enumerate(uniq):
            scale = f32(dt / (intervals[ix] - intervals[ix - 1]))
            lst = lsg[:, ix - 1, :]                # [BL, SIG]
            s = lst[:, 1:D + 1] * scale            # [BL, D]
            cv = lst[:, D + 1:] * scale            # [BL, P]
            C = np.zeros((BL, D, D), np.float32)
            bb = np.repeat(b, len(i0))
            np.add.at(C, (bb, np.tile(i0, BL), np.tile(i1, BL)),
                      cv[bb, np.tile(np.arange(len(i0)), BL)])
            np.add.at(C, (bb, np.tile(i1, BL), np.tile(i0, BL)),
                      -cv[bb, np.tile(np.arange(len(i0)), BL)])
            for c2 in range(2):
                blo = c2 * 16 + b16
                for d in range(D):
                    prow = d * 16 + b16
                    for e in range(D):
                        cblk[c, si, c2, prow, e * 16 + b16] = C[blo, d, e]
                    cblk[c, si, c2, prow, 128 + b16] = s[blo, d]
    y0 = (x0 @ W1.T + b1).astype(np.float32)       # [B, H]
    return dt, sl1, sl2, nS, cblk, y0


# ------------------------------------------------------------- bass program
def _build(nS, sl1, sl2):
    """Two independent 16-batch chains per core, interleaved so their serial
    dependency chains fill each other's engine gaps (the feval is latency-
    bound: every engine is <50% occupied on a single 32-batch chain).
    With 16-wide batch all 8 d-tangents fit in 128 partitions, so the
    transpose and the C-contraction are ONE op each.  bf16 matmuls (f32
    PSUM), Act restricted to the sigmoid_and_others table set."""
    import concourse.bass as bass
    import concourse.mybir as mybir
    from concourse.tile import TileContext

    f32 = mybir.dt.float32
    bf16 = mybir.dt.bfloat16
    AF = mybir.ActivationFunctionType
    OP = mybir.AluOpType
    nc = bass.Bass()
    B2 = 16          # batch per chain

    # wb cols: wv0ta 0:64 | wv1ta 64:128 | wvota 128:640 | w2ta 640:650
    #          | ident 650:714   (rows 0:64 weights, row 64 bias)
    WBC = 714
    wb = nc.declare_dram_parameter("wb", [H + 1, WBC], bf16, isOutput=False)
    y0p = nc.declare_dram_parameter("y0", [H, BL], f32, isOutput=False)
    cb = nc.declare_dram_parameter("cb", [128, nS, 2, 144], bf16,
                                   isOutput=False)
    probs = nc.declare_dram_parameter("probs", [BL, LABEL], f32,
                                      isOutput=True)

    with TileContext(nc) as tc:
        with tc.tile_pool(name="const", bufs=1) as cp, \
             tc.tile_pool(name="work", bufs=4) as wp, \
             tc.tile_pool(name="psA", bufs=1, space="PSUM") as pA, \
             tc.tile_pool(name="psW", bufs=4, space="PSUM") as pW, \
             tc.tile_pool(name="psV", bufs=2, space="PSUM") as pV, \
             tc.tile_pool(name="psT", bufs=1, space="PSUM") as pT:

            s_wb = cp.tile([H + 1, WBC], bf16)
            s_cb = cp.tile([128, nS, 2, 144], bf16)
            nc.sync.dma_start(s_wb[:], wb[:])
            nc.sync.dma_start(s_cb[:], cb[:])
            s_wv0ta = s_wb[:, 0:64]
            s_wv1ta = s_wb[:, 64:128]
            s_wv0t = s_wb[0:H, 0:64]
            s_wv1t = s_wb[0:H, 64:128]
            s_w2ta = s_wb[:, 640:650]
            s_id = s_wb[0:H, 650:714]

            ycur = cp.tile([H, BL], f32)
            nc.sync.dma_start(ycur[:], y0p[:])
            yfin = cp.tile([H + 1, BL], bf16)
            nc.vector.memset(yfin[H:H + 1, :], 1.0)

            chains = []
            for c2 in range(2):
                ctx = dict(
                    c2=c2,
                    ycur=ycur[:, c2 * B2:(c2 + 1) * B2],
                    ycast=cp.tile([H + 1, B2], bf16, name=f"ycast{c2}"),
                    ymidb=cp.tile([H + 1, B2], bf16, name=f"ymidb{c2}"),
                    h1c=cp.tile([H + 1, B2], bf16, name=f"h1c{c2}"),
                    h2c=cp.tile([H + 1, B2], bf16, name=f"h2c{c2}"),
                )
                for t in ("ycast", "ymidb", "h1c", "h2c"):
                    nc.vector.memset(ctx[t][H:H + 1, :], 1.0)
                chains.append(ctx)

            def feval(cx, y_bf, s_i, tag):
                c2 = cx["c2"]
                h1c, h2c = cx["h1c"], cx["h2c"]

                def silu_layer(ps, h_out_bf, sfx):
                    sg = wp.tile([H, B2], f32, tag="sg" + sfx)
                    nc.scalar.activation(sg[:], ps[:], AF.Sigmoid)
                    # bf16 h directly on the critical path; the derivative
                    # silu' = sg*(1 + a - h) takes 2 DVE ops off-path.
                    nc.vector.tensor_tensor(h_out_bf, ps[:], sg[:],
                                            op=OP.mult)
                    w = wp.tile([H, B2], f32, tag="w" + sfx)
                    nc.vector.scalar_tensor_tensor(w[:], ps[:], 1.0,
                                                   h_out_bf, op0=OP.add,
                                                   op1=OP.subtract)
                    dv = wp.tile([H, B2], f32, tag="dv" + sfx)
                    nc.vector.tensor_tensor(dv[:], w[:], sg[:], op=OP.mult)
                    return dv

                ps1 = pA.tile([H, B2], f32, tag="pa")
                nc.tensor.matmul(ps1[:], s_wv0ta, y_bf, start=True, stop=True)
                d1 = silu_layer(ps1, h1c[0:H, :], "1" + str(c2))
                ps2 = pA.tile([H, B2], f32, tag="pa")
                nc.tensor.matmul(ps2[:], s_wv1ta, h1c[:],
                                 start=True, stop=True)
                d2 = silu_layer(ps2, h2c[0:H, :], "2" + str(c2))

                ps3 = pW.tile([H, 8 * B2], f32, tag="pw")
                for d in range(D):
                    nc.tensor.matmul(ps3[:, B2 * d:B2 * d + B2],
                                     s_wb[:, 128 + 64 * d:192 + 64 * d],
                                     h2c[:], start=True, stop=True)
                xt = wp.tile([H, 8 * B2], bf16, tag="xt" + str(c2))
                nc.scalar.activation(xt[:], ps3[:], AF.Tanh)
                sq = wp.tile([H, 8 * B2], f32, tag="sq" + str(c2))
                nc.scalar.activation(sq[:], xt[:], AF.Square)
                # tp = 1 - X^2 off the critical path (P waits only on psg)
                tp = wp.tile([H, 8 * B2], f32, tag="tp" + str(c2))
                nc.vector.tensor_scalar(tp[:], sq[:], -1.0, 1.0,
                                        op0=OP.mult, op1=OP.add)
                # XW route: fold Wv0 into the C-contraction so no transpose
                # or PSUM round-trip sits on the critical path.
                # XW[(d,b), a] = sum_u X[b,d,u] Wv0t[u,a]  (lhsT = xt itself)
                # psd1[a,(e,b)] = sum_(d,b') XW[(d,b'),a] CB[(d,b'),(e,b)]
                psw = pW.tile([128, H], f32, tag="pw")
                nc.tensor.matmul(psw[:], xt[:], s_wv0t, start=True, stop=True)
                xwb = wp.tile([128, H], bf16, tag="xwb" + str(c2))
                nc.vector.tensor_copy(xwb[:], psw[:])
                psd1 = pW.tile([H, 8 * B2], f32, tag="pw")
                nc.tensor.matmul(psd1[:], xwb[:], s_cb[:, s_i, c2, 0:128],
                                 start=True, stop=True)
                # s-term via the transpose route, fully off the critical path
                pt0 = pT.tile([128, H], bf16, tag="pt")
                nc.tensor.transpose(pt0[:], xt[:], s_id)
                xtr = wp.tile([128, H], bf16, tag="xtr" + str(c2))
                nc.scalar.copy(xtr[:], pt0[:])
                psvs = pV.tile([H, B2], f32, tag="pv")
                nc.tensor.matmul(psvs[:], xtr[:], s_cb[:, s_i, c2, 128:144],
                                 start=True, stop=True)
                dh1 = wp.tile([H, 8 * B2], bf16, tag="dh1" + str(c2))
                nc.vector.tensor_tensor(
                    dh1[:].rearrange("p (e b) -> p e b", e=D),
                    psd1[:].rearrange("p (e b) -> p e b", e=D),
                    d1[:].unsqueeze(1).broadcast_to([H, D, B2]), op=OP.mult)
                psd2 = pW.tile([H, 8 * B2], f32, tag="pw")
                nc.tensor.matmul(psd2[:], s_wv1t, dh1[:],
                                 start=True, stop=True)
                dh2 = wp.tile([H, 8 * B2], bf16, tag="dh2" + str(c2))
                nc.vector.tensor_tensor(
                    dh2[:].rearrange("p (e b) -> p e b", e=D),
                    psd2[:].rearrange("p (e b) -> p e b", e=D),
                    d2[:].unsqueeze(1).broadcast_to([H, D, B2]), op=OP.mult)
                psg = pW.tile([H, 8 * B2], f32, tag="pw")
                for e in range(D):
                    nc.tensor.matmul(psg[:, B2 * e:B2 * e + B2],
                                     s_wb[0:H, 128 + 64 * e:192 + 64 * e],
                                     dh2[:, B2 * e:B2 * e + B2],
                                     start=True, stop=True)
                # P gets a 9th block holding the s-term (copied off-path from
                # psvs) so the reduce itself produces the complete drive.
                P = wp.tile([H, 9 * B2], f32, tag="P" + str(c2))
                nc.scalar.copy(P[:, 8 * B2:9 * B2], psvs[:])
                nc.vector.tensor_tensor(P[:, 0:8 * B2], psg[:], tp[:],
                                        op=OP.mult)
                dtk = wp.tile([H, B2], f32, tag=tag + str(c2))
                nc.vector.tensor_reduce(
                    dtk[:], P[:].rearrange("p (e b) -> p b e", e=D + 1),
                    axis=mybir.AxisListType.X, op=OP.add)
                return dtk

            for cx in chains:
                nc.scalar.copy(cx["ycast"][0:H, :], cx["ycur"])
            for k in range(N_STEPS):
                d1s = []
                for cx in chains:
                    d1s.append(feval(cx, cx["ycast"][:], sl1[k], "dtk1"))
                d2s = []
                for cx, dt1 in zip(chains, d1s):
                    nc.vector.tensor_tensor(cx["ymidb"][0:H, :], cx["ycur"],
                                            dt1[:], op=OP.add)
                    d2s.append(feval(cx, cx["ymidb"][:], sl2[k], "dtk2"))
                for cx, dt1, dt2 in zip(chains, d1s, d2s):
                    nc.vector.tensor_tensor(dt1[:], dt1[:], dt2[:], op=OP.add)
                    # bf16 ycast written first (reads the pre-update ycur),
                    # then the f32 master update — next ps1 waits only ycast.
                    nc.vector.scalar_tensor_tensor(cx["ycast"][0:H, :],
                                                   dt1[:], 0.5, cx["ycur"],
                                                   op0=OP.mult, op1=OP.add)
                    nc.vector.scalar_tensor_tensor(cx["ycur"], dt1[:], 0.5,
                                                   cx["ycur"],
                                                   op0=OP.mult, op1=OP.add)

            nc.vector.tensor_copy(yfin[0:H, :], ycur[:])
            pslg = pA.tile([BL, LABEL], f32, tag="pa")
            nc.tensor.matmul(pslg[:], yfin[:], s_w2ta, start=True, stop=True)
            mx = wp.tile([BL, 1], f32, tag="mx")
            nc.vector.tensor_reduce(mx[:], pslg[:], axis=mybir.AxisListType.X,
                                    op=OP.max, negate=True)
            lgs = wp.tile([BL, LABEL], f32, tag="lgs")
            nc.vector.tensor_scalar(lgs[:], pslg[:], mx[:], None, op0=OP.add)
            ex = wp.tile([BL, LABEL], f32, tag="ex")
            nc.scalar.activation(ex[:], lgs[:], AF.Exp)
            sm = wp.tile([BL, 1], f32, tag="sm")
            nc.vector.tensor_reduce(sm[:], ex[:], axis=mybir.AxisListType.X,
                                    op=OP.add)
            rp = wp.tile([BL, 1], f32, tag="rp")
            nc.vector.reciprocal(rp[:], sm[:])
            pr = wp.tile([BL, LABEL], f32, tag="pr")
            nc.vector.tensor_scalar(pr[:], ex[:], rp[:], None, op0=OP.mult)
            nc.sync.dma_start(probs[:], pr[:])
    return nc


LAST_EXEC_NS = None
_LAST = {}


def _make_runner(nc, n_cores):
    """Mirror concourse.bass2jax.run_bass_via_pjrt, but jit ONCE and return a
    reusable call closure (run_bass_via_pjrt re-traces + recompiles every
    invocation, which costs ~1.5s per call)."""
    import jax
    from jax.experimental.shard_map import shard_map
    from jax.sharding import Mesh, PartitionSpec
    from concourse import bass2jax, mybir

    try:
        jax.config.update("jax_compilation_cache_dir", "/tmp/jax_neff_cache")
        jax.config.update("jax_persistent_cache_min_compile_time_secs", 1.0)
    except Exception:
        pass
    bass2jax.install_neuronx_cc_hook()
    partition_name = (nc.partition_id_tensor.name
                      if nc.partition_id_tensor else None)
    in_names, out_names, out_avals, zero_outs = [], [], [], []
    for alloc in nc.m.functions[0].allocations:
        if not isinstance(alloc, mybir.MemoryLocationSet):
            continue
        name = alloc.memorylocations[0].name
        if alloc.kind == "ExternalInput":
            if name != partition_name:
                in_names.append(name)
        elif alloc.kind == "ExternalOutput":
            shape = tuple(alloc.tensor_shape)
            dtype = mybir.dt.np(alloc.dtype)
            out_names.append(name)
            out_avals.append(jax.core.ShapedArray(shape, dtype))
            zero_outs.append(np.zeros(shape, dtype))
    n_params = len(in_names)
    n_outs = len(out_avals)
    all_in = (list(in_names) + list(out_names)
              + ([partition_name] if partition_name else []))
    donate = tuple(range(n_params, n_params + n_outs))

    def _body(*args):
        operands = list(args)
        if partition_name is not None:
            operands.append(bass2jax.partition_id_tensor())
        outs = bass2jax._bass_exec_p.bind(
            *operands, out_avals=tuple(out_avals), in_names=tuple(all_in),
            out_names=tuple(out_names), lowering_input_output_aliases=(),
            sim_require_finite=True, sim_require_nnan=True, nc=nc)
        return tuple(outs)

    devices = jax.devices()[:n_cores]
    assert len(devices) == n_cores
    mesh = Mesh(np.asarray(devices), ("core",))
    in_specs = (PartitionSpec("core"),) * (n_params + n_outs)
    out_specs = (PartitionSpec("core"),) * n_outs
    sharded = jax.jit(
        shard_map(_body, mesh=mesh, in_specs=in_specs, out_specs=out_specs,
                  check_rep=False),
        donate_argnums=donate, keep_unused=True)
    in_shardings = jax.sharding.NamedSharding(mesh, PartitionSpec("core"))

    def stage(in_maps):
        """Concatenate per-core inputs and push to devices once."""
        per_core = [[np.asarray(m[name]) for name in in_names]
                    for m in in_maps]
        concat_in = [np.concatenate([per_core[c][i] for c in range(n_cores)],
                                    axis=0) for i in range(n_params)]
        return [jax.device_put(a, in_shardings) for a in concat_in]

    def call(staged):
        concat_zeros = [np.zeros((n_cores * z.shape[0], *z.shape[1:]),
                                 z.dtype) for z in zero_outs]
        out_arrs = sharded(*staged, *concat_zeros)
        jax.block_until_ready(out_arrs)
        return [
            {name: np.asarray(out_arrs[i]).reshape(
                n_cores, *out_avals[i].shape)[c]
             for i, name in enumerate(out_names)}
            for c in range(n_cores)]
    return stage, call


def rerun():
    """Re-execute the last compiled kernel (for timing warm runs)."""
    res = _LAST["call"](_LAST["staged"])
    return np.concatenate([np.asarray(res[c]["probs"])
                           for c in range(N_CORES)], axis=0)


def _run_device(inputs):
    global LAST_EXEC_NS
    _apply_compat_patches()

    f32 = np.float32
    ts = inputs["ts"].astype(f32)
    intervals = inputs["intervals"].astype(f32)
    logsig = inputs["logsig"].astype(f32)
    x0 = inputs["x0"].astype(f32)
    dt, sl1, sl2, nS, cblk, y0 = _host_prep(
        ts, intervals, logsig, x0, inputs["pairs"],
        inputs["W1"].astype(f32), inputs["b1"].astype(f32))

    nc = _build(nS, sl1, sl2)

    import ml_dtypes
    bf16 = ml_dtypes.bfloat16
    blob = np.zeros((H + 1, 714), f32)
    blob[0:H, 0:64] = inputs["Wv0"].astype(f32).T
    blob[H, 0:64] = inputs["bv0"].astype(f32)
    blob[0:H, 64:128] = inputs["Wv1"].astype(f32).T
    blob[H, 64:128] = inputs["bv1"].astype(f32)
    blob[:, 128:640] = np.vstack([inputs["Wvo"].astype(f32).T,
                                  inputs["bvo"].astype(f32)[None, :]])
    blob[:, 640:650] = np.vstack([inputs["W2"].astype(f32).T,
                                  inputs["b2"].astype(f32)[None, :]])
    blob[0:H, 650:714] = np.eye(H, dtype=f32)
    blob_b = blob.astype(bf16)

    in_maps = []
    for c in range(N_CORES):
        in_maps.append({
            "wb": blob_b,
            "y0": np.ascontiguousarray(y0[c * BL:(c + 1) * BL].T),
            "cb": np.ascontiguousarray(
                cblk[c].transpose(2, 0, 1, 3)).astype(bf16),
        })
    stage, call = _make_runner(nc, N_CORES)
    staged = stage(in_maps)
    _LAST.clear()
    _LAST.update(nc=nc, in_maps=in_maps, stage=stage, call=call,
                 staged=staged, full=True)
    res = call(staged)
    return np.concatenate([np.asarray(res[c]["probs"])
                           for c in range(N_CORES)], axis=0)


# ---------------------------------------------------------------- fallback
def _host_ode(inputs):
    f32 = np.float32
    ts = inputs["ts"].astype(f32); intervals = inputs["intervals"].astype(f32)
    logsig = inputs["logsig"].astype(f32); x0 = inputs["x0"].astype(f32)
    pairs = inputs["pairs"]
    W1, b1 = inputs["W1"].astype(f32), inputs["b1"].astype(f32)
    Wv0, bv0 = inputs["Wv0"].astype(f32), inputs["bv0"].astype(f32)
    Wv1, bv1 = inputs["Wv1"].astype(f32), inputs["bv1"].astype(f32)
    Wvo, bvo = inputs["Wvo"].astype(f32), inputs["bvo"].astype(f32)
    B, Dd = x0.shape
    t0, t1 = f32(ts[0]), f32(ts[-1])
    dt = f32((t1 - t0) / N_STEPS)
    times = (t0 + dt * np.arange(N_STEPS, dtype=f32)).astype(f32)
    i0 = pairs[:, 0] - 1; i1 = pairs[:, 1] - 1
    y = (x0 @ W1.T + b1).astype(f32)

    def func(t, y):
        idx = int(np.clip(np.searchsorted(intervals, t), 1, intervals.shape[0] - 1))
        lst = logsig[:, idx - 1, :]
        a1 = y @ Wv0.T + bv0; s1 = 1 / (1 + np.exp(-a1)); h1 = a1 * s1
        d1 = s1 * (1 + a1 * (1 - s1))
        a2 = h1 @ Wv1.T + bv1; s2 = 1 / (1 + np.exp(-a2)); h2 = a2 * s2
        d2 = s2 * (1 + a2 * (1 - s2))
        vf = np.tanh(h2 @ Wvo.T + bvo); tp = 1 - vf * vf
        vfr = vf.reshape(B, Dd, H)
        dA1 = vfr @ Wv0.T; dH1 = d1[:, None, :] * dA1
        dA2 = dH1 @ Wv1.T; dH2 = d2[:, None, :] * dA2
        dA3 = dH2 @ Wvo.T
        J = (tp[:, None, :] * dA3).reshape(B, Dd, Dd, H)
        s = lst[:, 1:Dd + 1]; c = lst[:, Dd + 1:]
        lie = J[:, i0, i1, :] - J[:, i1, i0, :]
        drive = np.einsum('bd,bdh->bh', s, vfr) + np.einsum('bp,bph->bh', c, lie)
        return (drive / f32(intervals[idx] - intervals[idx - 1])).astype(f32)

    for k in range(N_STEPS):
        t = times[k]
        k1 = func(t, y); k2 = func(f32(t + dt), y + dt * k1)
        y = (y + f32(0.5) * dt * (k1 + k2)).astype(f32)
    logits = y @ inputs["W2"].astype(f32).T + inputs["b2"].astype(f32)
    m = logits.max(axis=1, keepdims=True)
    e = np.exp(logits - m)
    return (e / e.sum(axis=1, keepdims=True)).astype(f32)




def _host_ode_fast(inputs):
    """Heun scan with the C-contraction applied before the JVP chain:
    4 matmuls of K=64,N<=512 per eval instead of the 8x larger dA3."""
    f32 = np.float32
    ts = inputs["ts"].astype(f32); intervals = inputs["intervals"].astype(f32)
    logsig = inputs["logsig"].astype(f32); x0 = inputs["x0"].astype(f32)
    pairs = inputs["pairs"]
    W1, b1 = inputs["W1"].astype(f32), inputs["b1"].astype(f32)
    Wv0, bv0 = inputs["Wv0"].astype(f32), inputs["bv0"].astype(f32)
    Wv1, bv1 = inputs["Wv1"].astype(f32), inputs["bv1"].astype(f32)
    Wvo, bvo = inputs["Wvo"].astype(f32), inputs["bvo"].astype(f32)
    B = x0.shape[0]
    t0, t1 = f32(ts[0]), f32(ts[-1])
    dt = f32((t1 - t0) / N_STEPS)
    times = (t0 + dt * np.arange(N_STEPS, dtype=f32)).astype(f32)
    i0 = pairs[:, 0].astype(np.int64) - 1
    i1 = pairs[:, 1].astype(np.int64) - 1
    npair = len(i0)
    Wvor = Wvo.reshape(D, H, H)          # [e, h, v]
    bvor = bvo.reshape(D, H)
    y = (x0 @ W1.T + b1).astype(f32)     # [B, H]

    def feval(y, idx):
        lst = logsig[:, idx - 1, :]
        scale = f32(dt / (intervals[idx] - intervals[idx - 1]))
        s = lst[:, 1:D + 1] * scale      # [B, D]
        cv = lst[:, D + 1:] * scale      # [B, P]
        C = np.zeros((B, D, D), f32)
        bb = np.repeat(np.arange(B), npair)
        pp = np.tile(np.arange(npair), B)
        np.add.at(C, (bb, i0[pp], i1[pp]), cv[bb, pp])
        np.add.at(C, (bb, i1[pp], i0[pp]), -cv[bb, pp])
        a1 = y @ Wv0.T + bv0
        s1 = 1 / (1 + np.exp(-a1)); h1 = a1 * s1; d1 = s1 * (1 + a1 * (1 - s1))
        a2 = h1 @ Wv1.T + bv1
        s2 = 1 / (1 + np.exp(-a2)); h2 = a2 * s2; d2 = s2 * (1 + a2 * (1 - s2))
        X = np.tanh(np.einsum('bv,ehv->beh', h2, Wvor) + bvor)   # [B, e, h]
        tp = 1.0 - X * X
        V = np.einsum('bde,bdu->beu', C, X)                      # [B, e, u]
        dA1 = V @ Wv0.T
        dH1 = d1[:, None, :] * dA1
        dA2 = dH1 @ Wv1.T
        U = d2[:, None, :] * dA2                                 # [B, e, v]
        G = np.einsum('bev,ehv->beh', U, Wvor)
        drive = (tp * G).sum(axis=1) + np.einsum('bd,bdh->bh', s, X)
        return drive.astype(f32)

    nI = intervals.shape[0] - 1
    for k in range(N_STEPS):
        idx1 = int(np.clip(np.searchsorted(intervals, times[k]), 1, nI))
        idx2 = int(np.clip(np.searchsorted(intervals, f32(times[k] + dt)), 1, nI))
        k1 = feval(y, idx1)
        k2 = feval(y + k1, idx2)
        y = (y + f32(0.5) * (k1 + k2)).astype(f32)
    return y


def _device_classifier(yT, W2, b2):
    """softmax(W2 @ y + b2) on 8 NeuronCores, batch-sharded."""
    _apply_compat_patches()
    import concourse.bass as bass
    import concourse.mybir as mybir
    from concourse.tile import TileContext
    from concourse.bass_utils import run_bass_kernel_spmd

    B = yT.shape[0]
    L = W2.shape[0]
    f32 = mybir.dt.float32
    AF = mybir.ActivationFunctionType
    OP = mybir.AluOpType

    nc = bass.Bass()
    yw_in = nc.declare_dram_parameter("yw", [H + 1, BL + L], f32, isOutput=False)
    pr_out = nc.declare_dram_parameter("probs", [BL, L], f32, isOutput=True)

    with TileContext(nc) as tc:
        with tc.tile_pool(name="sb", bufs=1) as pool, \
             tc.tile_pool(name="ps", bufs=1, space="PSUM") as pp:
            yw = pool.tile([H + 1, BL + L], f32)
            nc.sync.dma_start(yw[:], yw_in[:])
            ps = pp.tile([BL, L], f32)
            nc.tensor.matmul(ps[:], yw[:, 0:BL], yw[:, BL:BL + L],
                             start=True, stop=True)
            pr = pool.tile([BL, L], f32)
            nc.scalar.copy(pr[:], ps[:])
            nc.sync.dma_start(pr_out[:], pr[:])

    w_aug = np.vstack([W2.T.astype(np.float32),
                       b2.astype(np.float32)[None, :]])
    in_maps = []
    for c in range(N_CORES):
        ysh = yT[c * BL:(c + 1) * BL].T
        y_aug = np.vstack([ysh, np.ones((1, BL), np.float32)])
        in_maps.append({"yw": np.ascontiguousarray(
            np.hstack([y_aug, w_aug]))})
    stage, call = _make_runner(nc, N_CORES)
    staged = stage(in_maps)
    _LAST.clear()
    _LAST.update(nc=nc, in_maps=in_maps, stage=stage, call=call,
                 staged=staged, full=False)
    res = call(staged)
    logits = np.concatenate([np.asarray(res[c]["probs"])
                             for c in range(N_CORES)], axis=0)
    m = logits.max(axis=1, keepdims=True)
    e = np.exp(logits - m)
    return (e / e.sum(axis=1, keepdims=True)).astype(np.float32)


def kernel(**inputs):
    import os
    inputs = {k: np.asarray(v) for k, v in inputs.items()}
    if not os.environ.get("BASS_NO_FULL_ODE"):
        try:
            return _run_device(inputs)
        except Exception:
            import traceback; traceback.print_exc()
    try:
        y = _host_ode_fast(inputs)
    except Exception:
        import traceback; traceback.print_exc()
        return _host_ode(inputs)
    try:
        return _device_classifier(y, inputs["W2"].astype(np.float32),
                                  inputs["b2"].astype(np.float32))
    except Exception:
        import traceback; traceback.print_exc()
        logits = y @ inputs["W2"].astype(np.float32).T + inputs["b2"].astype(np.float32)
        m = logits.max(axis=1, keepdims=True)
        e = np.exp(logits - m)
        return (e / e.sum(axis=1, keepdims=True)).astype(np.float32)



# revision 56
# speedup vs baseline: 1.0021x; 1.0021x over previous
"""LogNeuralCDE on 8 NeuronCores, batch-sharded (32 items/core).

Whole Heun scan runs on-device. Math: for each func eval, the Lie-bracket
contraction sum_p c_p (J[i0,i1]-J[i1,i0]) is rewritten as sum_{d,e} C[b,d,e]
J[b,d,e,:] with C antisymmetric, and C is contracted against the tangent
basis X_d = vf rows BEFORE the JVP chain (linearity), so the chain runs on 8
e-tangents and the 512-wide Wvo matmul happens blockwise (8x smaller).
The s-term rides along as a 9th block through the same block-diag matmul.
Per-step C matrices (scaled by dt/denom) are host-built and preloaded.
"""

import numpy as np

N_CORES = 8
N_STEPS = 32
BL = 32          # batch per core
D = 8
H = 64
SIG = 37
LABEL = 10

_PATCHED = False


def _apply_compat_patches():
    """This container's neuronx-cc allows at most ONE sync wait per
    instruction (setupSyncWait: 'Too many sync wait commands').  Two fixes:
    1. TileContext's final drain carries the whole global-clock wait list —
       split it one-wait-per-NoOp before a bare drain.
    2. The tile scheduler emits compute instructions with 2 waits — rewrite
       the BIR JSON just before compile, moving excess waits onto NoOps
       inserted ahead of the instruction on the same engine.
    """
    global _PATCHED
    if _PATCHED:
        return
    _PATCHED = True
    import json
    from concourse import bass2jax, mybir
    from concourse.tile import TileContext
    from concourse.vector_clock import ScopedClock

    def _drain_and_barrier(self, tick_clock, wait_clock):
        nop_inst = self.nc.sync.nop(nofuse=True, hint="tile_final_wait")
        wait_clock.add_sem_waits(
            nop_inst.ins, ScopedClock({None: tick_clock.global_clock}))
        si = nop_inst.ins.sync_info
        waits = list(si.on_wait) if si is not None else []
        if len(waits) > 1:
            si.on_wait = waits[:1]
            for w in waits[1:]:
                extra = self.nc.sync.nop(nofuse=True, hint="tile_final_wait_x")
                extra.ins.sync_info = mybir.SyncInfo(on_wait=[w], on_update=[])
        self.nc.sync.drain()
        self.nc.all_engine_barrier()
        assert self.sems is not None
        popped = self.nc._tile_sem_poison_stack.pop()
        assert popped is self._sem_poison
        self.nc.clear_and_free_semaphores(list(self.sems.allocated().values()))
        self.nc.all_engine_barrier()

    TileContext._drain_and_barrier = _drain_and_barrier

    _orig_cbk = bass2jax.compile_bir_kernel

    def _split_excess_waits(bir_bytes):
        d = json.loads(bir_bytes)
        changed = False
        for fn in d.get("functions", []):
            for blk in fn.get("blocks", []):
                out = []
                ctr = 0
                for inst in blk.get("instructions", []):
                    si = inst.get("sync_info")
                    if si:
                        waits = si.get("on_wait") or []
                        if len(waits) > 1:
                            for w in waits[:-1]:
                                ctr += 1
                                out.append({
                                    "debug": inst.get("debug", 0),
                                    "engine": inst["engine"],
                                    "ins": [], "outs": [],
                                    "name": f"{inst['name']}_xw{ctr}",
                                    "opcode": "NoOp",
                                    "sync_info": {"on_update": [],
                                                  "on_wait": [w]},
                                })
                            si["on_wait"] = waits[-1:]
                            changed = True
                    out.append(inst)
                    if si:
                        ups = si.get("on_update") or []
                        if len(ups) > 1:
                            si["on_update"] = ups[:1]
                            for u in ups[1:]:
                                ctr += 1
                                out.append({
                                    "debug": inst.get("debug", 0),
                                    "engine": inst["engine"],
                                    "ins": [], "outs": [],
                                    "name": f"{inst['name']}_xu{ctr}",
                                    "opcode": "NoOp",
                                    "sync_info": {"on_update": [u],
                                                  "on_wait": []},
                                })
                            changed = True
                blk["instructions"] = out
        return json.dumps(d).encode() if changed else bir_bytes

    def _patched_cbk(bir_bytes, *a, **k):
        return _orig_cbk(_split_excess_waits(bir_bytes), *a, **k)

    bass2jax.compile_bir_kernel = _patched_cbk


# ---------------------------------------------------------------- host math
def _host_prep(ts, intervals, logsig, x0, pairs, W1, b1):
    f32 = np.float32
    B = x0.shape[0]
    t0, t1 = f32(ts[0]), f32(ts[-1])
    dt = f32((t1 - t0) / N_STEPS)
    times = (t0 + dt * np.arange(N_STEPS, dtype=f32)).astype(f32)
    nI = intervals.shape[0] - 1

    def eidx(t):
        return int(np.clip(np.searchsorted(intervals, t), 1, nI))

    idx1 = [eidx(times[k]) for k in range(N_STEPS)]
    idx2 = [eidx(f32(times[k] + dt)) for k in range(N_STEPS)]
    uniq = sorted(set(idx1) | set(idx2))
    pos = {ix: i for i, ix in enumerate(uniq)}
    sl1 = [pos[i] for i in idx1]
    sl2 = [pos[i] for i in idx2]
    nS = len(uniq)

    i0 = pairs[:, 0].astype(np.int64) - 1
    i1 = pairs[:, 1].astype(np.int64) - 1

    # Cblk[core, s, chain, 128, 144]: per 16-batch chain, rows (d, b16),
    # cols e*16+b16 | s-term at 128+b16.  All 8 d fit in 128 partitions.
    cblk = np.zeros((N_CORES, nS, 2, 128, 144), np.float32)
    b = np.arange(BL)
    b16 = np.arange(16)
    for c in range(N_CORES):
        lsg = logsig[c * BL:(c + 1) * BL]          # [BL, nI, SIG]
        for si, ix in enumerate(uniq):
            scale = f32(dt / (intervals[ix] - intervals[ix - 1]))
            lst = lsg[:, ix - 1, :]                # [BL, SIG]
            s = lst[:, 1:D + 1] * scale            # [BL, D]
            cv = lst[:, D + 1:] * scale            # [BL, P]
            C = np.zeros((BL, D, D), np.float32)
            bb = np.repeat(b, len(i0))
            np.add.at(C, (bb, np.tile(i0, BL), np.tile(i1, BL)),
                      cv[bb, np.tile(np.arange(len(i0)), BL)])
            np.add.at(C, (bb, np.tile(i1, BL), np.tile(i0, BL)),
                      -cv[bb, np.tile(np.arange(len(i0)), BL)])
            for c2 in range(2):
                blo = c2 * 16 + b16
                for d in range(D):
                    prow = d * 16 + b16
                    for e in range(D):
                        cblk[c, si, c2, prow, e * 16 + b16] = C[blo, d, e]
                    cblk[c, si, c2, prow, 128 + b16] = s[blo, d]
    y0 = (x0 @ W1.T + b1).astype(np.float32)       # [B, H]
    return dt, sl1, sl2, nS, cblk, y0


# ------------------------------------------------------------- bass program
def _build(nS, sl1, sl2):
    """Two independent 16-batch chains per core, interleaved so their serial
    dependency chains fill each other's engine gaps (the feval is latency-
    bound: every engine is <50% occupied on a single 32-batch chain).
    With 16-wide batch all 8 d-tangents fit in 128 partitions, so the
    transpose and the C-contraction are ONE op each.  bf16 matmuls (f32
    PSUM), Act restricted to the sigmoid_and_others table set."""
    import concourse.bass as bass
    import concourse.mybir as mybir
    from concourse.tile import TileContext

    f32 = mybir.dt.float32
    bf16 = mybir.dt.bfloat16
    AF = mybir.ActivationFunctionType
    OP = mybir.AluOpType
    nc = bass.Bass()
    B2 = 16          # batch per chain

    # wb cols: wv0ta 0:64 | wv1ta 64:128 | wvota 128:640 | w2ta 640:650
    #          | ident 650:714   (rows 0:64 weights, row 64 bias)
    WBC = 714
    wb = nc.declare_dram_parameter("wb", [H + 1, WBC], bf16, isOutput=False)
    y0p = nc.declare_dram_parameter("y0", [H, BL], f32, isOutput=False)
    cb = nc.declare_dram_parameter("cb", [128, nS, 2, 144], bf16,
                                   isOutput=False)
    probs = nc.declare_dram_parameter("probs", [BL, LABEL], f32,
                                      isOutput=True)

    with TileContext(nc) as tc:
        with tc.tile_pool(name="const", bufs=1) as cp, \
             tc.tile_pool(name="work", bufs=4) as wp, \
             tc.tile_pool(name="psA", bufs=1, space="PSUM") as pA, \
             tc.tile_pool(name="psW", bufs=4, space="PSUM") as pW, \
             tc.tile_pool(name="psV", bufs=2, space="PSUM") as pV, \
             tc.tile_pool(name="psT", bufs=1, space="PSUM") as pT:

            s_wb = cp.tile([H + 1, WBC], bf16)
            s_cb = cp.tile([128, nS, 2, 144], bf16)
            nc.sync.dma_start(s_wb[:], wb[:])
            nc.sync.dma_start(s_cb[:], cb[:])
            s_wv0ta = s_wb[:, 0:64]
            s_wv1ta = s_wb[:, 64:128]
            s_wv0t = s_wb[0:H, 0:64]
            s_wv1t = s_wb[0:H, 64:128]
            s_w2ta = s_wb[:, 640:650]
            s_id = s_wb[0:H, 650:714]

            ycur = cp.tile([H, BL], f32)
            nc.sync.dma_start(ycur[:], y0p[:])
            yfin = cp.tile([H + 1, BL], bf16)
            nc.vector.memset(yfin[H:H + 1, :], 1.0)

            chains = []
            for c2 in range(2):
                ctx = dict(
                    c2=c2,
                    ycur=ycur[:, c2 * B2:(c2 + 1) * B2],
                    ycast=cp.tile([H + 1, B2], bf16, name=f"ycast{c2}"),
                    ymidb=cp.tile([H + 1, B2], bf16, name=f"ymidb{c2}"),
                    h1c=cp.tile([H + 1, B2], bf16, name=f"h1c{c2}"),
                    h2c=cp.tile([H + 1, B2], bf16, name=f"h2c{c2}"),
                )
                for t in ("ycast", "ymidb", "h1c", "h2c"):
                    nc.vector.memset(ctx[t][H:H + 1, :], 1.0)
                chains.append(ctx)

            def feval(cx, y_bf, s_i, tag):
                c2 = cx["c2"]
                h1c, h2c = cx["h1c"], cx["h2c"]

                def silu_layer(ps, h_out_bf, sfx):
                    sg = wp.tile([H, B2], f32, tag="sg" + sfx)
                    nc.scalar.activation(sg[:], ps[:], AF.Sigmoid)
                    # bf16 h directly on the critical path; the derivative
                    # silu' = sg*(1 + a - h) takes 2 DVE ops off-path.
                    nc.vector.tensor_tensor(h_out_bf, ps[:], sg[:],
                                            op=OP.mult)
                    w = wp.tile([H, B2], f32, tag="w" + sfx)
                    nc.vector.scalar_tensor_tensor(w[:], ps[:], 1.0,
                                                   h_out_bf, op0=OP.add,
                                                   op1=OP.subtract)
                    dv = wp.tile([H, B2], f32, tag="dv" + sfx)
                    nc.vector.tensor_tensor(dv[:], w[:], sg[:], op=OP.mult)
                    return dv

                ps1 = pA.tile([H, B2], f32, tag="pa")
                nc.tensor.matmul(ps1[:], s_wv0ta, y_bf, start=True, stop=True)
                d1 = silu_layer(ps1, h1c[0:H, :], "1" + str(c2))
                ps2 = pA.tile([H, B2], f32, tag="pa")
                nc.tensor.matmul(ps2[:], s_wv1ta, h1c[:],
                                 start=True, stop=True)
                d2 = silu_layer(ps2, h2c[0:H, :], "2" + str(c2))

                ps3 = pW.tile([H, 8 * B2], f32, tag="pw")
                for d in range(D):
                    nc.tensor.matmul(ps3[:, B2 * d:B2 * d + B2],
                                     s_wb[:, 128 + 64 * d:192 + 64 * d],
                                     h2c[:], start=True, stop=True)
                xt = wp.tile([H, 8 * B2], bf16, tag="xt" + str(c2))
                nc.scalar.activation(xt[:], ps3[:], AF.Tanh)
                sq = wp.tile([H, 8 * B2], f32, tag="sq" + str(c2))
                nc.scalar.activation(sq[:], xt[:], AF.Square)
                # tp = 1 - X^2 off the critical path (P waits only on psg)
                tp = wp.tile([H, 8 * B2], f32, tag="tp" + str(c2))
                nc.vector.tensor_scalar(tp[:], sq[:], -1.0, 1.0,
                                        op0=OP.mult, op1=OP.add)
                # XW route: fold Wv0 into the C-contraction so no transpose
                # or PSUM round-trip sits on the critical path.
                # XW[(d,b), a] = sum_u X[b,d,u] Wv0t[u,a]  (lhsT = xt itself)
                # psd1[a,(e,b)] = sum_(d,b') XW[(d,b'),a] CB[(d,b'),(e,b)]
                psw = pW.tile([128, H], f32, tag="pw")
                nc.tensor.matmul(psw[:], xt[:], s_wv0t, start=True, stop=True)
                xwb = wp.tile([128, H], bf16, tag="xwb" + str(c2))
                nc.vector.tensor_copy(xwb[:], psw[:])
                psd1 = pW.tile([H, 8 * B2], f32, tag="pw")
                nc.tensor.matmul(psd1[:], xwb[:], s_cb[:, s_i, c2, 0:128],
                                 start=True, stop=True)
                # s-term via the transpose route, fully off the critical path
                pt0 = pT.tile([128, H], bf16, tag="pt")
                nc.tensor.transpose(pt0[:], xt[:], s_id)
                xtr = wp.tile([128, H], bf16, tag="xtr" + str(c2))
                nc.scalar.copy(xtr[:], pt0[:])
                psvs = pV.tile([H, B2], f32, tag="pv")
                nc.tensor.matmul(psvs[:], xtr[:], s_cb[:, s_i, c2, 128:144],
                                 start=True, stop=True)
                dh1 = wp.tile([H, 8 * B2], bf16, tag="dh1" + str(c2))
                nc.vector.tensor_tensor(
                    dh1[:].rearrange("p (e b) -> p e b", e=D),
                    psd1[:].rearrange("p (e b) -> p e b", e=D),
                    d1[:].unsqueeze(1).broadcast_to([H, D, B2]), op=OP.mult)
                psd2 = pW.tile([H, 8 * B2], f32, tag="pw")
                nc.tensor.matmul(psd2[:], s_wv1t, dh1[:],
                                 start=True, stop=True)
                dh2 = wp.tile([H, 8 * B2], bf16, tag="dh2" + str(c2))
                nc.vector.tensor_tensor(
                    dh2[:].rearrange("p (e b) -> p e b", e=D),
                    psd2[:].rearrange("p (e b) -> p e b", e=D),
                    d2[:].unsqueeze(1).broadcast_to([H, D, B2]), op=OP.mult)
                psg = pW.tile([H, 8 * B2], f32, tag="pw")
                for e in range(D):
                    nc.tensor.matmul(psg[:, B2 * e:B2 * e + B2],
                                     s_wb[0:H, 128 + 64 * e:192 + 64 * e],
                                     dh2[:, B2 * e:B2 * e + B2],
                                     start=True, stop=True)
                # P gets a 9th block holding the s-term (copied off-path from
                # psvs) so the reduce itself produces the complete drive.
                P = wp.tile([H, 9 * B2], f32, tag="P" + str(c2))
                nc.scalar.copy(P[:, 8 * B2:9 * B2], psvs[:])
                nc.vector.tensor_tensor(P[:, 0:8 * B2], psg[:], tp[:],
                                        op=OP.mult)
                dtk = wp.tile([H, B2], f32, tag=tag + str(c2))
                nc.vector.tensor_reduce(
                    dtk[:], P[:].rearrange("p (e b) -> p b e", e=D + 1),
                    axis=mybir.AxisListType.X, op=OP.add)
                return dtk

            for cx in chains:
                nc.scalar.copy(cx["ycast"][0:H, :], cx["ycur"])
            for k in range(N_STEPS):
                d1s = []
                for cx in chains:
                    d1s.append(feval(cx, cx["ycast"][:], sl1[k], "dtk1"))
                d2s = []
                for cx, dt1 in zip(chains, d1s):
                    nc.vector.tensor_tensor(cx["ymidb"][0:H, :], cx["ycur"],
                                            dt1[:], op=OP.add)
                    d2s.append(feval(cx, cx["ymidb"][:], sl2[k], "dtk2"))
                for cx, dt1, dt2 in zip(chains, d1s, d2s):
                    nc.vector.tensor_tensor(dt1[:], dt1[:], dt2[:], op=OP.add)
                    # bf16 ycast written first (reads the pre-update ycur),
                    # then the f32 master update — next ps1 waits only ycast.
                    nc.vector.scalar_tensor_tensor(cx["ycast"][0:H, :],
                                                   dt1[:], 0.5, cx["ycur"],
                                                   op0=OP.mult, op1=OP.add)
                    nc.vector.scalar_tensor_tensor(cx["ycur"], dt1[:], 0.5,
                                                   cx["ycur"],
                                                   op0=OP.mult, op1=OP.add)

            nc.vector.tensor_copy(yfin[0:H, :], ycur[:])
            pslg = pA.tile([BL, LABEL], f32, tag="pa")
            nc.tensor.matmul(pslg[:], yfin[:], s_w2ta, start=True, stop=True)
            mx = wp.tile([BL, 1], f32, tag="mx")
            nc.vector.tensor_reduce(mx[:], pslg[:], axis=mybir.AxisListType.X,
                                    op=OP.max, negate=True)
            lgs = wp.tile([BL, LABEL], f32, tag="lgs")
            nc.vector.tensor_scalar(lgs[:], pslg[:], mx[:], None, op0=OP.add)
            ex = wp.tile([BL, LABEL], f32, tag="ex")
            nc.scalar.activation(ex[:], lgs[:], AF.Exp)
            sm = wp.tile([BL, 1], f32, tag="sm")
            nc.vector.tensor_reduce(sm[:], ex[:], axis=mybir.AxisListType.X,
                                    op=OP.add)
            rp = wp.tile([BL, 1], f32, tag="rp")
            nc.vector.reciprocal(rp[:], sm[:])
            pr = wp.tile([BL, LABEL], f32, tag="pr")
            nc.vector.tensor_scalar(pr[:], ex[:], rp[:], None, op0=OP.mult)
            nc.sync.dma_start(probs[:], pr[:])
    return nc


LAST_EXEC_NS = None
_LAST = {}


def _make_runner(nc, n_cores):
    """Mirror concourse.bass2jax.run_bass_via_pjrt, but jit ONCE and return a
    reusable call closure (run_bass_via_pjrt re-traces + recompiles every
    invocation, which costs ~1.5s per call)."""
    import jax
    from jax.experimental.shard_map import shard_map
    from jax.sharding import Mesh, PartitionSpec
    from concourse import bass2jax, mybir

    try:
        jax.config.update("jax_compilation_cache_dir", "/tmp/jax_neff_cache")
        jax.config.update("jax_persistent_cache_min_compile_time_secs", 1.0)
    except Exception:
        pass
    bass2jax.install_neuronx_cc_hook()
    partition_name = (nc.partition_id_tensor.name
                      if nc.partition_id_tensor else None)
    in_names, out_names, out_avals, zero_outs = [], [], [], []
    for alloc in nc.m.functions[0].allocations:
        if not isinstance(alloc, mybir.MemoryLocationSet):
            continue
        name = alloc.memorylocations[0].name
        if alloc.kind == "ExternalInput":
            if name != partition_name:
                in_names.append(name)
        elif alloc.kind == "ExternalOutput":
            shape = tuple(alloc.tensor_shape)
            dtype = mybir.dt.np(alloc.dtype)
            out_names.append(name)
            out_avals.append(jax.core.ShapedArray(shape, dtype))
            zero_outs.append(np.zeros(shape, dtype))
    n_params = len(in_names)
    n_outs = len(out_avals)
    all_in = (list(in_names) + list(out_names)
              + ([partition_name] if partition_name else []))
    donate = tuple(range(n_params, n_params + n_outs))

    def _body(*args):
        operands = list(args)
        if partition_name is not None:
            operands.append(bass2jax.partition_id_tensor())
        outs = bass2jax._bass_exec_p.bind(
            *operands, out_avals=tuple(out_avals), in_names=tuple(all_in),
            out_names=tuple(out_names), lowering_input_output_aliases=(),
            sim_require_finite=True, sim_require_nnan=True, nc=nc)
        return tuple(outs)

    devices = jax.devices()[:n_cores]
    assert len(devices) == n_cores
    mesh = Mesh(np.asarray(devices), ("core",))
    in_specs = (PartitionSpec("core"),) * (n_params + n_outs)
    out_specs = (PartitionSpec("core"),) * n_outs
    sharded = jax.jit(
        shard_map(_body, mesh=mesh, in_specs=in_specs, out_specs=out_specs,
                  check_rep=False),
        donate_argnums=donate, keep_unused=True)
    in_shardings = jax.sharding.NamedSharding(mesh, PartitionSpec("core"))

    def stage(in_maps):
        """Concatenate per-core inputs and push to devices once."""
        per_core = [[np.asarray(m[name]) for name in in_names]
                    for m in in_maps]
        concat_in = [np.concatenate([per_core[c][i] for c in range(n_cores)],
                                    axis=0) for i in range(n_params)]
        return [jax.device_put(a, in_shardings) for a in concat_in]

    def call(staged):
        concat_zeros = [np.zeros((n_cores * z.shape[0], *z.shape[1:]),
                                 z.dtype) for z in zero_outs]
        out_arrs = sharded(*staged, *concat_zeros)
        jax.block_until_ready(out_arrs)
        return [
            {name: np.asarray(out_arrs[i]).reshape(
                n_cores, *out_avals[i].shape)[c]
             for i, name in enumerate(out_names)}
            for c in range(n_cores)]
    return stage, call


def rerun():
    """Re-execute the last compiled kernel (for timing warm runs)."""
    res = _LAST["call"](_LAST["staged"])
    return np.concatenate([np.asarray(res[c]["probs"])
                           for c in range(N_CORES)], axis=0)


def _run_device(inputs):
    global LAST_EXEC_NS
    _apply_compat_patches()

    f32 = np.float32
    ts = inputs["ts"].astype(f32)
    intervals = inputs["intervals"].astype(f32)
    logsig = inputs["logsig"].astype(f32)
    x0 = inputs["x0"].astype(f32)
    dt, sl1, sl2, nS, cblk, y0 = _host_prep(
        ts, intervals, logsig, x0, inputs["pairs"],
        inputs["W1"].astype(f32), inputs["b1"].astype(f32))

    nc = _build(nS, sl1, sl2)

    import ml_dtypes
    bf16 = ml_dtypes.bfloat16
    blob = np.zeros((H + 1, 714), f32)
    blob[0:H, 0:64] = inputs["Wv0"].astype(f32).T
    blob[H, 0:64] = inputs["bv0"].astype(f32)
    blob[0:H, 64:128] = inputs["Wv1"].astype(f32).T
    blob[H, 64:128] = inputs["bv1"].astype(f32)
    blob[:, 128:640] = np.vstack([inputs["Wvo"].astype(f32).T,
                                  inputs["bvo"].astype(f32)[None, :]])
    blob[:, 640:650] = np.vstack([inputs["W2"].astype(f32).T,
                                  inputs["b2"].astype(f32)[None, :]])
    blob[0:H, 650:714] = np.eye(H, dtype=f32)
    blob_b = blob.astype(bf16)

    in_maps = []
    for c in range(N_CORES):
        in_maps.append({
            "wb": blob_b,
            "y0": np.ascontiguousarray(y0[c * BL:(c + 1) * BL].T),
            "cb": np.ascontiguousarray(
                cblk[c].transpose(2, 0, 1, 3)).astype(bf16),
        })
    stage, call = _make_runner(nc, N_CORES)
    staged = stage(in_maps)
    _LAST.clear()
    _LAST.update(nc=nc, in_maps=in_maps, stage=stage, call=call,
                 staged=staged, full=True)
    res = call(staged)
    return np.concatenate([np.asarray(res[c]["probs"])
                           for c in range(N_CORES)], axis=0)


# ---------------------------------------------------------------- fallback
def _host_ode(inputs):
    f32 = np.float32
    ts = inputs["ts"].astype(f32); intervals = inputs["intervals"].astype(f32)
    logsig = inputs["logsig"].astype(f32); x0 = inputs["x0"].astype(f32)
    pairs = inputs["pairs"]
    W1, b1 = inputs["W1"].astype(f32), inputs["b1"].astype(f32)
    Wv0, bv0 = inputs["Wv0"].astype(f32), inputs["bv0"].astype(f32)
    Wv1, bv1 = inputs["Wv1"].astype(f32), inputs["bv1"].astype(f32)
    Wvo, bvo = inputs["Wvo"].astype(f32), inputs["bvo"].astype(f32)
    B, Dd = x0.shape
    t0, t1 = f32(ts[0]), f32(ts[-1])
    dt = f32((t1 - t0) / N_STEPS)
    times = (t0 + dt * np.arange(N_STEPS, dtype=f32)).astype(f32)
    i0 = pairs[:, 0] - 1; i1 = pairs[:, 1] - 1
    y = (x0 @ W1.T + b1).astype(f32)

    def func(t, y):
        idx = int(np.clip(np.searchsorted(intervals, t), 1, intervals.shape[0] - 1))
        lst = logsig[:, idx - 1, :]
        a1 = y @ Wv0.T + bv0; s1 = 1 / (1 + np.exp(-a1)); h1 = a1 * s1
        d1 = s1 * (1 + a1 * (1 - s1))
        a2 = h1 @ Wv1.T + bv1; s2 = 1 / (1 + np.exp(-a2)); h2 = a2 * s2
        d2 = s2 * (1 + a2 * (1 - s2))
        vf = np.tanh(h2 @ Wvo.T + bvo); tp = 1 - vf * vf
        vfr = vf.reshape(B, Dd, H)
        dA1 = vfr @ Wv0.T; dH1 = d1[:, None, :] * dA1
        dA2 = dH1 @ Wv1.T; dH2 = d2[:, None, :] * dA2
        dA3 = dH2 @ Wvo.T
        J = (tp[:, None, :] * dA3).reshape(B, Dd, Dd, H)
        s = lst[:, 1:Dd + 1]; c = lst[:, Dd + 1:]
        lie = J[:, i0, i1, :] - J[:, i1, i0, :]
        drive = np.einsum('bd,bdh->bh', s, vfr) + np.einsum('bp,bph->bh', c, lie)
        return (drive / f32(intervals[idx] - intervals[idx - 1])).astype(f32)

    for k in range(N_STEPS):
        t = times[k]
        k1 = func(t, y); k2 = func(f32(t + dt), y + dt * k1)
        y = (y + f32(0.5) * dt * (k1 + k2)).astype(f32)
    logits = y @ inputs["W2"].astype(f32).T + inputs["b2"].astype(f32)
    m = logits.max(axis=1, keepdims=True)
    e = np.exp(logits - m)
    return (e / e.sum(axis=1, keepdims=True)).astype(f32)




def _host_ode_fast(inputs):
    """Heun scan with the C-contraction applied before the JVP chain:
    4 matmuls of K=64,N<=512 per eval instead of the 8x larger dA3."""
    f32 = np.float32
    ts = inputs["ts"].astype(f32); intervals = inputs["intervals"].astype(f32)
    logsig = inputs["logsig"].astype(f32); x0 = inputs["x0"].astype(f32)
    pairs = inputs["pairs"]
    W1, b1 = inputs["W1"].astype(f32), inputs["b1"].astype(f32)
    Wv0, bv0 = inputs["Wv0"].astype(f32), inputs["bv0"].astype(f32)
    Wv1, bv1 = inputs["Wv1"].astype(f32), inputs["bv1"].astype(f32)
    Wvo, bvo = inputs["Wvo"].astype(f32), inputs["bvo"].astype(f32)
    B = x0.shape[0]
    t0, t1 = f32(ts[0]), f32(ts[-1])
    dt = f32((t1 - t0) / N_STEPS)
    times = (t0 + dt * np.arange(N_STEPS, dtype=f32)).astype(f32)
    i0 = pairs[:, 0].astype(np.int64) - 1
    i1 = pairs[:, 1].astype(np.int64) - 1
    npair = len(i0)
    Wvor = Wvo.reshape(D, H, H)          # [e, h, v]
    bvor = bvo.reshape(D, H)
    y = (x0 @ W1.T + b1).astype(f32)     # [B, H]

    def feval(y, idx):
        lst = logsig[:, idx - 1, :]
        scale = f32(dt / (intervals[idx] - intervals[idx - 1]))
        s = lst[:, 1:D + 1] * scale      # [B, D]
        cv = lst[:, D + 1:] * scale      # [B, P]
        C = np.zeros((B, D, D), f32)
        bb = np.repeat(np.arange(B), npair)
        pp = np.tile(np.arange(npair), B)
        np.add.at(C, (bb, i0[pp], i1[pp]), cv[bb, pp])
        np.add.at(C, (bb, i1[pp], i0[pp]), -cv[bb, pp])
        a1 = y @ Wv0.T + bv0
        s1 = 1 / (1 + np.exp(-a1)); h1 = a1 * s1; d1 = s1 * (1 + a1 * (1 - s1))
        a2 = h1 @ Wv1.T + bv1
        s2 = 1 / (1 + np.exp(-a2)); h2 = a2 * s2; d2 = s2 * (1 + a2 * (1 - s2))
        X = np.tanh(np.einsum('bv,ehv->beh', h2, Wvor) + bvor)   # [B, e, h]
        tp = 1.0 - X * X
        V = np.einsum('bde,bdu->beu', C, X)                      # [B, e, u]
        dA1 = V @ Wv0.T
        dH1 = d1[:, None, :] * dA1
        dA2 = dH1 @ Wv1.T
        U = d2[:, None, :] * dA2                                 # [B, e, v]
        G = np.einsum('bev,ehv->beh', U, Wvor)
        drive = (tp * G).sum(axis=1) + np.einsum('bd,bdh->bh', s, X)
        return drive.astype(f32)

    nI = intervals.shape[0] - 1
    for k in range(N_STEPS):
        idx1 = int(np.clip(np.searchsorted(intervals, times[k]), 1, nI))
        idx2 = int(np.clip(np.searchsorted(intervals, f32(times[k] + dt)), 1, nI))
        k1 = feval(y, idx1)
        k2 = feval(y + k1, idx2)
        y = (y + f32(0.5) * (k1 + k2)).astype(f32)
    return y


def _device_classifier(yT, W2, b2):
    """softmax(W2 @ y + b2) on 8 NeuronCores, batch-sharded."""
    _apply_compat_patches()
    import concourse.bass as bass
    import concourse.mybir as mybir
    from concourse.tile import TileContext
    from concourse.bass_utils import run_bass_kernel_spmd

    B = yT.shape[0]
    L = W2.shape[0]
    f32 = mybir.dt.float32
    AF = mybir.ActivationFunctionType
    OP = mybir.AluOpType

    nc = bass.Bass()
    yw_in = nc.declare_dram_parameter("yw", [H + 1, BL + L], f32, isOutput=False)
    pr_out = nc.declare_dram_parameter("probs", [BL, L], f32, isOutput=True)

    with TileContext(nc) as tc:
        with tc.tile_pool(name="sb", bufs=1) as pool, \
             tc.tile_pool(name="ps", bufs=1, space="PSUM") as pp:
            yw = pool.tile([H + 1, BL + L], f32)
            nc.sync.dma_start(yw[:], yw_in[:])
            ps = pp.tile([BL, L], f32)
            nc.tensor.matmul(ps[:], yw[:, 0:BL], yw[:, BL:BL + L],
                             start=True, stop=True)
            pr = pool.tile([BL, L], f32)
            nc.scalar.copy(pr[:], ps[:])
            nc.sync.dma_start(pr_out[:], pr[:])

    w_aug = np.vstack([W2.T.astype(np.float32),
                       b2.astype(np.float32)[None, :]])
    in_maps = []
    for c in range(N_CORES):
        ysh = yT[c * BL:(c + 1) * BL].T
        y_aug = np.vstack([ysh, np.ones((1, BL), np.float32)])
        in_maps.append({"yw": np.ascontiguousarray(
            np.hstack([y_aug, w_aug]))})
    stage, call = _make_runner(nc, N_CORES)
    staged = stage(in_maps)
    _LAST.clear()
    _LAST.update(nc=nc, in_maps=in_maps, stage=stage, call=call,
                 staged=staged, full=False)
    res = call(staged)
    logits = np.concatenate([np.asarray(res[c]["probs"])
                             for c in range(N_CORES)], axis=0)
    m = logits.max(axis=1, keepdims=True)
    e = np.exp(logits - m)
    return (e / e.sum(axis=1, keepdims=True)).astype(np.float32)


def kernel(**inputs):
    import os
    inputs = {k: np.asarray(v) for k, v in inputs.items()}
    if not os.environ.get("BASS_NO_FULL_ODE"):
        try:
            return _run_device(inputs)
        except Exception:
            import traceback; traceback.print_exc()
    try:
        y = _host_ode_fast(inputs)
    except Exception:
        import traceback; traceback.print_exc()
        return _host_ode(inputs)
    try:
        return _device_classifier(y, inputs["W2"].astype(np.float32),
                                  inputs["b2"].astype(np.float32))
    except Exception:
        import traceback; traceback.print_exc()
        logits = y @ inputs["W2"].astype(np.float32).T + inputs["b2"].astype(np.float32)
        m = logits.max(axis=1, keepdims=True)
        e = np.exp(logits - m)
        return (e / e.sum(axis=1, keepdims=True)).astype(np.float32)

